# revision 60
# baseline (speedup 1.0000x reference)
"""Distributed Trainium2 kernel for nn_CONNECT_86964497809993 (TGN-style
GNN message passing: last-event aggregation + GRU memory update + community
incidence matmul), sharded over 8 NeuronCores.

Strategy: event routing ("last message per node") is integer index plumbing
done on the host during input sharding (per the sharding hint); nodes are
then re-partitioned across cores into a has-event set (full GRU pipeline)
and a no-event set (memory passthrough: only the community matmul needs
those rows). Per core:
  - gates   psum = [embs|embd|feat|tenc+mask|mem] @ [W_ih;W_hh]*8 computed
            with fp8e4 DoubleRow matmuls (2 K-tiles per instr, 0.5 cyc/row).
            Each 128-row K-chunk is a DR pair (W8, Wlo) sharing one
            stride-0-broadcast fp8 stream, where W8 = fp8(8W) and
            Wlo = fp8(8W - W8): the fp8 weight-residual kills the
            systematic per-column quantization bias that otherwise
            accumulates over the 100k-node community reduction.
  - mask    has-mask folded into the z-gate as an extra stream row with
            weight 240 (sigma(30) = 1 => passthrough), zero elementwise cost.
  - GRU     ACT: sigmoid/tanh at scale 1/8 (un-scales the 8x fp8 weights);
            DVE: r*hn, xn+ (psum-coupled), two blend ops + psum evacuation;
            Pool: the final blend add (off the critical chain, lag-3 tail).
  - comm    new_mem tiles PE-transposed (f16, via identity) into PSUM,
            evacuated by DVE tensor_copy, then f16-stationary x fp8e3-moving
            matmuls accumulate incidence^T partials; no-event nodes
            contribute via fp8e3 matmuls directly from memory tiles, spread
            through the pipeline to fill PE gaps. incidence/memory use
            float8e3 (e3m4: 4 mantissa bits) - their error enters a 100k-term
            reduction, and e4m3 would double it.
All streams are feature-major [feat, node] so every DMA moves >=1KB
contiguous runs per partition (full DMA bus rate) and memory is loaded
once. The whole emission is software-pipelined 6 deep across the five
in-order engine queues (gates/sigma -> tanh/blend -> transpose/evac ->
comm) with PE p-state warm-up at t=0 and progressive DMA slab sizes.
Community partials ([128,256] per core) are summed on the host."""

import numpy as np
import ml_dtypes

from concourse import bacc
import concourse.mybir as mybir
from concourse.tile import TileContext
from concourse.bass_utils import run_bass_kernel_spmd

N, E, C = 100000, 50000, 256
M, D, F, T = 128, 128, 128, 64
NCORES = 8
P = 128

f32 = mybir.dt.float32
f16 = mybir.dt.float16
f8 = mybir.dt.float8e4
f8e3 = mybir.dt.float8e3
A = mybir.AluOpType
AF = mybir.ActivationFunctionType
DR = mybir.MatmulPerfMode.DoubleRow

NP_E4 = ml_dtypes.float8_e4m3
NP_E3 = ml_dtypes.float8_e3m4
WS = 8.0                  # weight pre-scale (un-done by ACT scale=1/8)
ZBIG = 240.0              # mask weight: sigma(240/8) = 1.0
DMA_SLAB = 1024           # has1 DMA slab (columns)
SUB = 512                 # compute sub-slab (columns)

_COMPILED = {}            # (H1, H0) -> compiled program


def _q8(a):
    return np.asarray(a, np.float32).astype(NP_E4)


def _q8e3(a):
    return np.asarray(a, np.float32).astype(NP_E3)


def _build_program(H1, H0):
    T1 = H1 // P
    T0 = H0 // P
    KD = T + 2  # D-chunk contraction depth: 64 tenc + mask + bias
    nc = bacc.Bacc("TRN2", target_bir_lowering=False)

    X8 = nc.dram_tensor("X8", [P, 4, H1], f8, kind="ExternalInput")
    t8 = nc.dram_tensor("t8", [KD, H1], f8, kind="ExternalInput")
    mT = nc.dram_tensor("mT", [P, H1], f16, kind="ExternalInput")
    inc1 = nc.dram_tensor("inc1", [P, T1, 256], f8e3, kind="ExternalInput")
    m0 = nc.dram_tensor("m0", [P, T0, 128], f8e3, kind="ExternalInput")
    inc0 = nc.dram_tensor("inc0", [P, T0, 256], f8e3, kind="ExternalInput")
    WP = nc.dram_tensor("WP", [P, 13, 2, 128], f8, kind="ExternalInput")
    idt = nc.dram_tensor("idt", [P, P], f16, kind="ExternalInput")
    om = nc.dram_tensor("om", [P, H1], f16, kind="ExternalOutput")
    ocm = nc.dram_tensor("ocm", [P, 256], f32, kind="ExternalOutput")

    # X8 planes: 0=emb_s 1=emb_d 2=mem 3=feat; plane 4 = t8 (66-deep chunk).
    # r-gate runs without the weight-residual (numerically validated): its
    # k-tile pairs are adjacent X8 plane pairs (s,d) and (m,f) plus t8, so it
    # needs only 3 DoubleRow instructions. z/xn/hn keep (W8,Wlo) pairs on a
    # stride-0-duplicated stream.
    SEGS = [
        (1, [3, 4, 5, 6, 7], [0, 1, 3, 4, 2]),   # z   <- psum rz[:,1,:]
        (2, [8, 9, 10, 11], [0, 1, 3, 4]),       # xn  <- psum xn[:,0,:]
        (3, [12], [2]),                          # hn  <- psum xn[:,1,:]
    ]

    with TileContext(nc) as tc:
        with tc.tile_pool(name="const", bufs=1) as cpool, \
             tc.tile_pool(name="xp", bufs=3) as xpool, \
             tc.tile_pool(name="gp", bufs=3) as gpool, \
             tc.tile_pool(name="op", bufs=3) as opool, \
             tc.tile_pool(name="rzp", bufs=1, space="PSUM") as rzpool, \
             tc.tile_pool(name="xnp", bufs=2, space="PSUM") as xnpool, \
             tc.tile_pool(name="trp", bufs=1, space="PSUM") as trpool, \
             tc.tile_pool(name="cap", bufs=1, space="PSUM") as capool:

            wp_t = cpool.tile([P, 13, 2, 128], f8)
            nc.scalar.dma_start(wp_t[:, 0:3], WP[:, 0:3])   # r-gate slots first
            nc.scalar.dma_start(wp_t[:, 3:13], WP[:, 3:13])
            id_t = cpool.tile([P, P], f16)
            nc.scalar.dma_start(id_t[:], idt[:])
            comm = capool.tile([P, 256], f32)
            m0_t = cpool.tile([P, T0, 128], f8e3)
            inc0_t = cpool.tile([P, T0, 256], f8e3)

            # PE p-state warm-up: dependency-free matmuls on a zeroed scratch
            # tile keep the tensor engine continuously busy through the first
            # input DMAs so real gate matmuls start at full clock. Results land
            # in the comm psum bank, which the first real community matmul
            # resets via its start flag.
            warm = gpool.tile([P, 512], f16, tag="warm")
            nc.gpsimd.memset(warm[:], 0.0)
            for _ in range(20):
                nc.tensor.matmul(comm[:], warm[:, 0:128], warm[:, 256:512],
                                 start=True, stop=True, skip_group_check=True)

            # ---- Phase 1: has-event nodes ----
            # Fully software-pipelined across in-order engine queues:
            #   cycle k emits: gates(k) [PE], sigma(k) [ACT], tt/npre(k) [DVE],
            #   then the LAGGED stages: tanh(k-1) [ACT] (so sigma(k) is never
            #   queued behind a tanh that waits on DVE), blend(k-1) [Pool+DVE],
            #   transposes(k-3) [PE], evac(k-3) [DVE], comm(k-4) [PE], and a
            #   few phase-0 matmuls to fill PE gaps.
            h0_state = {"next": 0, "armed": False, "dma": False}

            def emit_h0(count):
                p = h0_state["next"]
                while count > 0 and p < T0:
                    nc.tensor.matmul(comm[:], m0_t[:, p, :], inc0_t[:, p, :],
                                     start=False, stop=False)
                    p += 1; count -= 1
                h0_state["next"] = p

            # Progressive slab sizes: small first slabs start PE early; later,
            # bigger transfers hide behind compute.
            slab_edges = [0]
            for w in (512, 512, 1024, 1024):
                if slab_edges[-1] + w < H1:
                    slab_edges.append(slab_edges[-1] + w)
            while slab_edges[-1] < H1:
                slab_edges.append(min(slab_edges[-1] + DMA_SLAB, H1))

            # Stage A: gate matmuls + sigma + tt/npre for sub-slab ss.
            def stage_a(ss):
                x_s, t8_s, mT_s, inc_s, o_s, g0, s0, w = (
                    ss["x"], ss["t8"], ss["mT"], ss["inc"], ss["o"],
                    ss["g0"], ss["s0"], ss["w"])
                rz_ps = rzpool.tile([P, 2, SUB], f32, tag="rz")
                xn_ps = xnpool.tile([P, 2, SUB], f32, tag="xn")
                halves = []
                for h0c in range(0, w, 256):
                    cs = slice(s0 + h0c, s0 + h0c + 256)
                    rhs = [x_s[:, pl, cs].unsqueeze(1).broadcast_to([P, 2, 256])
                           for pl in range(4)]
                    rhs.append(t8_s[:, cs].unsqueeze(1).broadcast_to([KD, 2, 256]))
                    halves.append((h0c, cs, rhs))
                # r/z for BOTH halves first so sigma's input is ready early;
                # r-gate: (s,d) and (m,f) adjacent-plane pairs + t8.
                for h0c, cs, rhs in halves:
                    dest = rz_ps[:, 0, h0c:h0c + 256]
                    nc.tensor.matmul(dest, wp_t[:, 0, :, :], x_s[:, 0:2, cs],
                                     start=True, stop=False, perf_mode=DR)
                    nc.tensor.matmul(dest, wp_t[:, 1, :, :], x_s[:, 2:4, cs],
                                     start=False, stop=False, perf_mode=DR)
                    nc.tensor.matmul(dest, wp_t[0:KD, 2, :, :], rhs[4],
                                     start=False, stop=True, perf_mode=DR)
                    d_idx, pairs, planes = SEGS[0]
                    dest = rz_ps[:, 1, h0c:h0c + 256]
                    for i, (j, pl) in enumerate(zip(pairs, planes)):
                        lhs = wp_t[0:KD, j, :, :] if pl == 4 else wp_t[:, j, :, :]
                        nc.tensor.matmul(dest, lhs, rhs[pl],
                                         start=(i == 0), stop=(i == len(pairs) - 1),
                                         perf_mode=DR)
                for h0c, cs, rhs in halves:
                    for d_idx, pairs, planes in SEGS[1:]:
                        dest = xn_ps[:, d_idx % 2, h0c:h0c + 256]
                        for i, (j, pl) in enumerate(zip(pairs, planes)):
                            lhs = wp_t[0:KD, j, :, :] if pl == 4 else wp_t[:, j, :, :]
                            nc.tensor.matmul(dest, lhs, rhs[pl],
                                             start=(i == 0), stop=(i == len(pairs) - 1),
                                             perf_mode=DR)
                rz = gpool.tile([P, 2, SUB], f16, tag="rzs")
                nc.scalar.activation(rz[:, :, 0:w], rz_ps[:, :, 0:w],
                                     AF.Sigmoid, scale=1.0 / WS)
                tt = gpool.tile([P, SUB], f16, tag="tt")
                nc.vector.tensor_tensor(tt[:, 0:w], rz[:, 0, 0:w],
                                        xn_ps[:, 1, 0:w], A.mult)
                npre = gpool.tile([P, SUB], f16, tag="npre")
                nc.vector.tensor_tensor(npre[:, 0:w], xn_ps[:, 0, 0:w],
                                        tt[:, 0:w], A.add)
                ss["rz"], ss["npre"] = rz, npre

            # Stage B: tanh + blend for sub-slab ss (one cycle after stage A).
            # The final sub-slab keeps its whole blend on DVE: the Pool hop
            # would add ~800ns to the drain's serial dependency chain.
            def stage_b(ss):
                w = ss["w"]
                n_t = gpool.tile([P, SUB], f16, tag="n")
                nc.scalar.activation(n_t[:, 0:w], ss["npre"][:, 0:w],
                                     AF.Tanh, scale=1.0 / WS)
                d_t = gpool.tile([P, SUB], f16, tag="d")
                nc.vector.tensor_tensor(d_t[:, 0:w],
                                        ss["mT"][:, ss["s0"]:ss["s0"] + w],
                                        n_t[:, 0:w], A.subtract)
                e_t = gpool.tile([P, SUB], f16, tag="e")
                nc.vector.tensor_tensor(e_t[:, 0:w], ss["rz"][:, 1, 0:w],
                                        d_t[:, 0:w], A.mult)
                o_eng = nc.vector if ss["final"] else nc.gpsimd
                o_eng.tensor_tensor(ss["o"][:, ss["s0"]:ss["s0"] + w],
                                    n_t[:, 0:w], e_t[:, 0:w], A.add)
                if ss["g0"] + ss["W"] >= H1:
                    # last slab: per-sub-slab stores shorten the drain tail
                    nc.sync.dma_start(om[:, ss["g0"] + ss["s0"]:ss["g0"] + ss["s0"] + w],
                                      ss["o"][:, ss["s0"]:ss["s0"] + w])
                elif ss["last_in_slab"]:
                    nc.sync.dma_start(om[:, ss["g0"]:ss["g0"] + ss["W"]], ss["o"][:])

            # Stage C: transposes + psum evacuation (lag 3).
            def stage_c(ss):
                nt = ss["w"] // P
                tr_ps = trpool.tile([P, 4, 128], f16, tag="tr")
                nmT = gpool.tile([P, 4, 128], f16, tag="nmT")
                for k in range(nt):
                    nc.tensor.transpose(tr_ps[:, k, :],
                                        ss["o"][:, ss["s0"] + P * k:ss["s0"] + P * (k + 1)],
                                        id_t[:])
                nc.vector.tensor_copy(nmT[:, 0:nt, :], tr_ps[:, 0:nt, :])
                ss["nmT"] = nmT

            # Stage D: community matmuls (lag 4). The last node-tile carries
            # the accumulation-group stop flag (phase-0 matmuls all precede it
            # in PE program order).
            def stage_d(ss):
                for k in range(ss["w"] // P):
                    t_idx = (ss["g0"] + ss["s0"]) // P + k
                    nc.tensor.matmul(comm[:], ss["nmT"][:, k, :],
                                     ss["inc"][:, ss["s0"] // P + k, :],
                                     start=(t_idx == 0), stop=(t_idx == T1 - 1))

            subs = []
            emitted = {"b": 0, "c": 0, "d": 0}

            def pump(k):
                # Run lagged stages for cycle k of the pipeline.
                if k - 1 >= 0 and k - 1 < len(subs):
                    stage_b(subs[k - 1]); emitted["b"] = k
                if k - 3 >= 0 and k - 3 < len(subs):
                    stage_c(subs[k - 3]); emitted["c"] = k - 3
                if k - 5 >= 0 and k - 5 < len(subs):
                    stage_d(subs[k - 5]); emitted["d"] = k - 5
                    if h0_state["armed"]:
                        emit_h0(4)

            kk = 0
            pend_inc = []
            for g0, g1 in zip(slab_edges[:-1], slab_edges[1:]):
                W = g1 - g0
                x_s = xpool.tile([P, 4, W], f8, tag="x")
                nc.sync.dma_start(x_s[:], X8[:, :, g0:g0 + W])
                t8_s = xpool.tile([KD, W], f8, tag="t8")
                nc.gpsimd.dma_start(t8_s[:], t8[:, g0:g0 + W])
                mT_s = xpool.tile([P, W], f16, tag="mT")
                nc.scalar.dma_start(mT_s[:], mT[:, g0:g0 + W])
                # inc isn't consumed until the lag-4 comm stage: defer its DMA
                # one slab so early bandwidth goes to the gate streams.
                inc_s = xpool.tile([P, W // P, 256], f8e3, tag="inc")
                pend_inc.append((inc_s, g0, W))
                if len(pend_inc) > 1:
                    i_s, ig0, iW = pend_inc.pop(0)
                    nc.gpsimd.dma_start(i_s[:], inc1[:, ig0 // P:(ig0 + iW) // P, :])
                o_s = opool.tile([P, W], f16, tag="o")
                if g0 == 3072:
                    nc.scalar.dma_start(m0_t[:], m0[:])
                    nc.scalar.dma_start(inc0_t[:], inc0[:])
                    h0_state["dma"] = True
                elif g0 >= 5120:
                    h0_state["armed"] = h0_state["dma"]

                for s0 in range(0, W, SUB):
                    w = min(SUB, W - s0)
                    subs.append(dict(x=x_s, t8=t8_s, mT=mT_s, inc=inc_s, o=o_s,
                                     g0=g0, s0=s0, W=W, w=w,
                                     last_in_slab=(s0 + w >= W),
                                     final=(g0 + s0 + w >= H1)))
                    stage_a(subs[kk])
                    pump(kk)
                    kk += 1

            for i_s, ig0, iW in pend_inc:
                nc.gpsimd.dma_start(i_s[:], inc1[:, ig0 // P:(ig0 + iW) // P, :])

            if not h0_state["dma"]:  # few-slab edge case: load phase-0 now
                nc.scalar.dma_start(m0_t[:], m0[:])
                nc.scalar.dma_start(inc0_t[:], inc0[:])

            # Drain the pipeline; phase-0 remainder goes first so the final
            # community matmul (stop flag) is the true end of the psum group.
            emit_h0(T0)
            for k in range(kk, kk + 6):
                pump(k)

            cm = gpool.tile([P, 256], f32, tag="cm")
            nc.scalar.activation(cm[:], comm[:], AF.Copy)
            nc.sync.dma_start(ocm[:], cm[:])

    nc.compile()
    return nc


def _get_program(H1, H0):
    key = (H1, H0)
    if key not in _COMPILED:
        _COMPILED[key] = _build_program(H1, H0)
    return _COMPILED[key]


def _ceil_to(x, q):
    return (x + q - 1) // q * q


def kernel(src, dst, t, last_update, event_feat, src_embeds, dst_embeds,
           nodes_memory, incidence, w_time, b_time, W_ih, W_hh, b_ih, b_hh):
    src = np.asarray(src); dst = np.asarray(dst); t = np.asarray(t)
    last_update = np.asarray(last_update)
    event_feat = np.asarray(event_feat, np.float32)
    src_embeds = np.asarray(src_embeds, np.float32)
    dst_embeds = np.asarray(dst_embeds, np.float32)
    nodes_memory = np.asarray(nodes_memory, np.float32)
    incidence = np.asarray(incidence, np.float32)
    w_time = np.asarray(w_time, np.float32); b_time = np.asarray(b_time, np.float32)
    W_ih = np.asarray(W_ih, np.float32); W_hh = np.asarray(W_hh, np.float32)
    b_ih = np.asarray(b_ih, np.float32); b_hh = np.asarray(b_hh, np.float32)

    # ---- Host routing: 'last' aggregation = stable-sort scatter (index-only) ----
    src_all = np.concatenate([src, dst])
    t_all = np.concatenate([t, t])
    perm = np.argsort(t_all, kind="stable")
    win = np.zeros(N, np.int64)
    win[src_all[perm]] = perm          # ascending rank; last write = newest event
    has = np.bincount(src_all, minlength=N) > 0

    nodes1 = np.where(has)[0]
    nodes0 = np.where(~has)[0]
    n1, n0 = len(nodes1), len(nodes0)
    h1c = (n1 + NCORES - 1) // NCORES
    h0c = (n0 + NCORES - 1) // NCORES
    H1 = _ceil_to(max(h1c, 1), 256)
    H0 = _ceil_to(max(h0c, 1), 256)
    T0 = H0 // P

    # Winner-event data for has-event nodes
    w1 = win[nodes1]
    dtw = (t_all[w1] - last_update[nodes1]).astype(np.float32)
    tenc = np.cos(dtw[:, None] * w_time[None, :] + b_time[None, :])  # [n1, T]
    lt = w1 < E
    w0 = np.where(lt, w1, w1 - E)
    emb_s = np.where(lt[:, None], src_embeds[w0], dst_embeds[w0])
    emb_d = np.where(lt[:, None], dst_embeds[w0], src_embeds[w0])
    feat = event_feat[w0]

    # ---- Replicated weights: fp8 DoubleRow pairs (W8, Wlo), pre-scaled by 8 ----
    bias = (b_ih + b_hh).astype(np.float32)
    chunks = {}  # name -> [128, 384] f32 weight rows
    chunks['A'] = W_ih[0:128]
    chunks['B'] = W_ih[128:256]
    chunks['C'] = W_ih[256:384]
    Dr = np.zeros((128, 384), np.float32)
    Dr[0:T] = W_ih[384:448]
    Dr[T + 1] = bias                      # bias lane (stream row = 1.0)
    chunks['D'] = Dr
    chunks['E'] = W_hh
    # Slot layout: r (slots 0-2, no residual): [A8|B8], [E8|C8], [D8|0];
    # z (3-7), xn (8-11), hn (12): (W8, Wlo) residual pairs per chunk.
    WPa = np.zeros((P, 13, 2, 128), np.float32)

    def _hi(cname, seg):
        wseg = chunks[cname][:, 128 * seg:128 * (seg + 1)] * WS
        if seg == 1 and cname == 'D':   # z-seg D chunk carries the has-mask row
            wseg = wseg.copy()
            wseg[T] = ZBIG
        return wseg, _q8(wseg).astype(np.float32)

    WPa[:, 0, 0, :] = _hi('A', 0)[1]
    WPa[:, 0, 1, :] = _hi('B', 0)[1]
    WPa[:, 1, 0, :] = _hi('E', 0)[1]
    WPa[:, 1, 1, :] = _hi('C', 0)[1]
    WPa[:, 2, 0, :] = _hi('D', 0)[1]
    for j, (seg, cname) in enumerate([(1, c) for c in "ABCDE"] +
                                     [(2, c) for c in "ABCD"] + [(2, 'E')], start=3):
        wseg, hi = _hi(cname, seg)
        WPa[:, j, 0, :] = hi
        WPa[:, j, 1, :] = _q8(wseg - hi).astype(np.float32)
    WP_v = WPa.astype(NP_E4)

    ident = np.eye(P, dtype=np.float16)

    nc = _get_program(H1, H0)

    in_maps = []
    core_n1 = []
    for c in range(NCORES):
        i0, i1 = c * h1c, min((c + 1) * h1c, n1)
        cn1 = max(i1 - i0, 0)
        core_n1.append((i0, i1))
        sl = slice(i0, i1)
        X8c = np.zeros((P, 4, H1), NP_E4)
        X8c[:, 0, :cn1] = _q8(emb_s[sl].T)
        X8c[:, 1, :cn1] = _q8(emb_d[sl].T)
        X8c[:, 2, :cn1] = _q8(nodes_memory[nodes1[sl]].T)
        X8c[:, 3, :cn1] = _q8(feat[sl].T)
        tpl = np.zeros((T + 2, H1), np.float32)
        tpl[0:T, :cn1] = tenc[sl].T
        tpl[T, cn1:] = 1.0                # mask row: 1 on padding columns
        tpl[T + 1, :] = 1.0               # bias lane
        t8c = _q8(tpl)
        mTc = np.zeros((P, H1), np.float16)
        mTc[:, :cn1] = nodes_memory[nodes1[sl]].T.astype(np.float16)
        inc1c = np.zeros((H1 // P, P, 256), NP_E3)
        inc1c.reshape(H1, 256)[:cn1] = _q8e3(incidence[nodes1[sl]])
        inc1c = np.ascontiguousarray(inc1c.transpose(1, 0, 2))

        j0, j1 = c * h0c, min((c + 1) * h0c, n0)
        cn0 = max(j1 - j0, 0)
        sl0 = nodes0[j0:j1]
        m0c = np.zeros((H0, 128), NP_E3)
        m0c[:cn0] = _q8e3(nodes_memory[sl0])
        m0c = np.ascontiguousarray(m0c.reshape(T0, P, 128).transpose(1, 0, 2))
        inc0c = np.zeros((H0, 256), NP_E3)
        inc0c[:cn0] = _q8e3(incidence[sl0])
        inc0c = np.ascontiguousarray(inc0c.reshape(T0, P, 256).transpose(1, 0, 2))

        in_maps.append(dict(X8=X8c, t8=t8c, mT=mTc, inc1=inc1c, m0=m0c, inc0=inc0c,
                            WP=WP_v, idt=ident))

    res = run_bass_kernel_spmd(nc, in_maps, core_ids=list(range(NCORES)))

    out = np.empty((N + C, M), np.float32)
    out[:N] = nodes_memory
    comm = np.zeros((M, C), np.float64)
    for c in range(NCORES):
        i0, i1 = core_n1[c]
        if i1 > i0:
            out[nodes1[i0:i1]] = res.results[c]["om"][:, :i1 - i0].T.astype(np.float32)
        comm += res.results[c]["ocm"]
    out[N:] = comm.T.astype(np.float32)
    return out


# revision 61
# speedup vs baseline: 1.0013x; 1.0013x over previous
"""Distributed Trainium2 kernel for nn_CONNECT_86964497809993 (TGN-style
GNN message passing: last-event aggregation + GRU memory update + community
incidence matmul), sharded over 8 NeuronCores.

Strategy: event routing ("last message per node") is integer index plumbing
done on the host during input sharding (per the sharding hint); nodes are
then re-partitioned across cores into a has-event set (full GRU pipeline)
and a no-event set (memory passthrough: only the community matmul needs
those rows). Per core:
  - gates   psum = [embs|embd|feat|tenc+mask|mem] @ [W_ih;W_hh]*8 computed
            with fp8e4 DoubleRow matmuls (2 K-tiles per instr, 0.5 cyc/row).
            Each 128-row K-chunk is a DR pair (W8, Wlo) sharing one
            stride-0-broadcast fp8 stream, where W8 = fp8(8W) and
            Wlo = fp8(8W - W8): the fp8 weight-residual kills the
            systematic per-column quantization bias that otherwise
            accumulates over the 100k-node community reduction.
  - mask    has-mask folded into the z-gate as an extra stream row with
            weight 240 (sigma(30) = 1 => passthrough), zero elementwise cost.
  - GRU     ACT: sigmoid/tanh at scale 1/8 (un-scales the 8x fp8 weights);
            DVE: r*hn, xn+ (psum-coupled), two blend ops + psum evacuation;
            Pool: the final blend add (off the critical chain, lag-3 tail).
  - comm    new_mem tiles PE-transposed (f16, via identity) into PSUM,
            evacuated by DVE tensor_copy, then f16-stationary x fp8e3-moving
            matmuls accumulate incidence^T partials; no-event nodes
            contribute via fp8e3 matmuls directly from memory tiles, spread
            through the pipeline to fill PE gaps. incidence/memory use
            float8e3 (e3m4: 4 mantissa bits) - their error enters a 100k-term
            reduction, and e4m3 would double it.
All streams are feature-major [feat, node] so every DMA moves >=1KB
contiguous runs per partition (full DMA bus rate) and memory is loaded
once. The whole emission is software-pipelined 6 deep across the five
in-order engine queues (gates/sigma -> tanh/blend -> transpose/evac ->
comm) with PE p-state warm-up at t=0 and progressive DMA slab sizes.
Community partials ([128,256] per core) are summed on the host."""

import numpy as np
import ml_dtypes

from concourse import bacc
import concourse.mybir as mybir
from concourse.tile import TileContext
from concourse.bass_utils import run_bass_kernel_spmd

N, E, C = 100000, 50000, 256
M, D, F, T = 128, 128, 128, 64
NCORES = 8
P = 128

f32 = mybir.dt.float32
f16 = mybir.dt.float16
f8 = mybir.dt.float8e4
f8e3 = mybir.dt.float8e3
A = mybir.AluOpType
AF = mybir.ActivationFunctionType
DR = mybir.MatmulPerfMode.DoubleRow

NP_E4 = ml_dtypes.float8_e4m3
NP_E3 = ml_dtypes.float8_e3m4
WS = 8.0                  # weight pre-scale (un-done by ACT scale=1/8)
ZBIG = 240.0              # mask weight: sigma(240/8) = 1.0
DMA_SLAB = 1024           # has1 DMA slab (columns)
SUB = 512                 # compute sub-slab (columns)

_COMPILED = {}            # (H1, H0) -> compiled program


def _q8(a):
    return np.asarray(a, np.float32).astype(NP_E4)


def _q8e3(a):
    return np.asarray(a, np.float32).astype(NP_E3)


def _build_program(H1, H0):
    T1 = H1 // P
    T0 = H0 // P
    KD = T + 2  # D-chunk contraction depth: 64 tenc + mask + bias
    nc = bacc.Bacc("TRN2", target_bir_lowering=False)

    X8 = nc.dram_tensor("X8", [P, 4, H1], f8, kind="ExternalInput")
    t8 = nc.dram_tensor("t8", [KD, H1], f8, kind="ExternalInput")
    mT = nc.dram_tensor("mT", [P, H1], f16, kind="ExternalInput")
    inc1 = nc.dram_tensor("inc1", [P, T1, 256], f8e3, kind="ExternalInput")
    m0 = nc.dram_tensor("m0", [P, T0, 128], f8e3, kind="ExternalInput")
    inc0 = nc.dram_tensor("inc0", [P, T0, 256], f8e3, kind="ExternalInput")
    WP = nc.dram_tensor("WP", [P, 13, 2, 128], f8, kind="ExternalInput")
    idt = nc.dram_tensor("idt", [P, P], f16, kind="ExternalInput")
    om = nc.dram_tensor("om", [P, H1], f16, kind="ExternalOutput")
    ocm = nc.dram_tensor("ocm", [P, 256], f32, kind="ExternalOutput")

    # X8 planes: 0=emb_s 1=emb_d 2=mem 3=feat; plane 4 = t8 (66-deep chunk).
    # r-gate runs without the weight-residual (numerically validated): its
    # k-tile pairs are adjacent X8 plane pairs (s,d) and (m,f) plus t8, so it
    # needs only 3 DoubleRow instructions. z/xn/hn keep (W8,Wlo) pairs on a
    # stride-0-duplicated stream.
    SEGS = [
        (1, [3, 4, 5, 6, 7], [0, 1, 3, 4, 2]),   # z   <- psum rz[:,1,:]
        (2, [8, 9, 10, 11], [0, 1, 3, 4]),       # xn  <- psum xn[:,0,:]
        (3, [12], [2]),                          # hn  <- psum xn[:,1,:]
    ]

    with TileContext(nc) as tc:
        with tc.tile_pool(name="const", bufs=1) as cpool, \
             tc.tile_pool(name="xp", bufs=3) as xpool, \
             tc.tile_pool(name="gp", bufs=3) as gpool, \
             tc.tile_pool(name="op", bufs=3) as opool, \
             tc.tile_pool(name="rzp", bufs=1, space="PSUM") as rzpool, \
             tc.tile_pool(name="xnp", bufs=2, space="PSUM") as xnpool, \
             tc.tile_pool(name="trp", bufs=1, space="PSUM") as trpool, \
             tc.tile_pool(name="cap", bufs=1, space="PSUM") as capool:

            wp_t = cpool.tile([P, 13, 2, 128], f8)
            nc.sync.dma_start(wp_t[:, 0:3], WP[:, 0:3])   # r-gate slots first
            nc.scalar.dma_start(wp_t[:, 3:13], WP[:, 3:13])
            id_t = cpool.tile([P, P], f16)
            nc.scalar.dma_start(id_t[:], idt[:])
            comm = capool.tile([P, 256], f32)
            m0_t = cpool.tile([P, T0, 128], f8e3)
            inc0_t = cpool.tile([P, T0, 256], f8e3)

            # PE p-state warm-up: dependency-free matmuls on a zeroed scratch
            # tile keep the tensor engine continuously busy through the first
            # input DMAs so real gate matmuls start at full clock. Results land
            # in the comm psum bank, which the first real community matmul
            # resets via its start flag.
            warm = gpool.tile([P, 512], f16, tag="warm")
            nc.gpsimd.memset(warm[:], 0.0)
            for _ in range(20):
                nc.tensor.matmul(comm[:], warm[:, 0:128], warm[:, 256:512],
                                 start=True, stop=True, skip_group_check=True)

            # ---- Phase 1: has-event nodes ----
            # Fully software-pipelined across in-order engine queues:
            #   cycle k emits: gates(k) [PE], sigma(k) [ACT], tt/npre(k) [DVE],
            #   then the LAGGED stages: tanh(k-1) [ACT] (so sigma(k) is never
            #   queued behind a tanh that waits on DVE), blend(k-1) [Pool+DVE],
            #   transposes(k-3) [PE], evac(k-3) [DVE], comm(k-4) [PE], and a
            #   few phase-0 matmuls to fill PE gaps.
            h0_state = {"next": 0, "armed": False, "dma": False}

            def emit_h0(count):
                p = h0_state["next"]
                while count > 0 and p < T0:
                    nc.tensor.matmul(comm[:], m0_t[:, p, :], inc0_t[:, p, :],
                                     start=False, stop=False)
                    p += 1; count -= 1
                h0_state["next"] = p

            # Progressive slab sizes: small first slabs start PE early; later,
            # bigger transfers hide behind compute.
            slab_edges = [0]
            for w in (512, 512, 1024, 1024):
                if slab_edges[-1] + w < H1:
                    slab_edges.append(slab_edges[-1] + w)
            while slab_edges[-1] < H1:
                slab_edges.append(min(slab_edges[-1] + DMA_SLAB, H1))

            # Stage A: gate matmuls + sigma + tt/npre for sub-slab ss.
            def stage_a(ss):
                x_s, t8_s, mT_s, inc_s, o_s, g0, s0, w = (
                    ss["x"], ss["t8"], ss["mT"], ss["inc"], ss["o"],
                    ss["g0"], ss["s0"], ss["w"])
                rz_ps = rzpool.tile([P, 2, SUB], f32, tag="rz")
                xn_ps = xnpool.tile([P, 2, SUB], f32, tag="xn")
                halves = []
                for h0c in range(0, w, 256):
                    cs = slice(s0 + h0c, s0 + h0c + 256)
                    rhs = [x_s[:, pl, cs].unsqueeze(1).broadcast_to([P, 2, 256])
                           for pl in range(4)]
                    rhs.append(t8_s[:, cs].unsqueeze(1).broadcast_to([KD, 2, 256]))
                    halves.append((h0c, cs, rhs))
                # r/z for BOTH halves first so sigma's input is ready early;
                # r-gate: (s,d) and (m,f) adjacent-plane pairs + t8.
                for h0c, cs, rhs in halves:
                    dest = rz_ps[:, 0, h0c:h0c + 256]
                    nc.tensor.matmul(dest, wp_t[:, 0, :, :], x_s[:, 0:2, cs],
                                     start=True, stop=False, perf_mode=DR)
                    nc.tensor.matmul(dest, wp_t[:, 1, :, :], x_s[:, 2:4, cs],
                                     start=False, stop=False, perf_mode=DR)
                    nc.tensor.matmul(dest, wp_t[0:KD, 2, :, :], rhs[4],
                                     start=False, stop=True, perf_mode=DR)
                    d_idx, pairs, planes = SEGS[0]
                    dest = rz_ps[:, 1, h0c:h0c + 256]
                    for i, (j, pl) in enumerate(zip(pairs, planes)):
                        lhs = wp_t[0:KD, j, :, :] if pl == 4 else wp_t[:, j, :, :]
                        nc.tensor.matmul(dest, lhs, rhs[pl],
                                         start=(i == 0), stop=(i == len(pairs) - 1),
                                         perf_mode=DR)
                for h0c, cs, rhs in halves:
                    for d_idx, pairs, planes in SEGS[1:]:
                        dest = xn_ps[:, d_idx % 2, h0c:h0c + 256]
                        for i, (j, pl) in enumerate(zip(pairs, planes)):
                            lhs = wp_t[0:KD, j, :, :] if pl == 4 else wp_t[:, j, :, :]
                            nc.tensor.matmul(dest, lhs, rhs[pl],
                                             start=(i == 0), stop=(i == len(pairs) - 1),
                                             perf_mode=DR)
                rz = gpool.tile([P, 2, SUB], f16, tag="rzs")
                nc.scalar.activation(rz[:, :, 0:w], rz_ps[:, :, 0:w],
                                     AF.Sigmoid, scale=1.0 / WS)
                tt = gpool.tile([P, SUB], f16, tag="tt")
                nc.vector.tensor_tensor(tt[:, 0:w], rz[:, 0, 0:w],
                                        xn_ps[:, 1, 0:w], A.mult)
                npre = gpool.tile([P, SUB], f16, tag="npre")
                nc.vector.tensor_tensor(npre[:, 0:w], xn_ps[:, 0, 0:w],
                                        tt[:, 0:w], A.add)
                ss["rz"], ss["npre"] = rz, npre

            # Stage B: tanh + blend for sub-slab ss (one cycle after stage A).
            # The final sub-slab keeps its whole blend on DVE: the Pool hop
            # would add ~800ns to the drain's serial dependency chain.
            def stage_b(ss):
                w = ss["w"]
                n_t = gpool.tile([P, SUB], f16, tag="n")
                nc.scalar.activation(n_t[:, 0:w], ss["npre"][:, 0:w],
                                     AF.Tanh, scale=1.0 / WS)
                d_t = gpool.tile([P, SUB], f16, tag="d")
                nc.vector.tensor_tensor(d_t[:, 0:w],
                                        ss["mT"][:, ss["s0"]:ss["s0"] + w],
                                        n_t[:, 0:w], A.subtract)
                e_t = gpool.tile([P, SUB], f16, tag="e")
                nc.vector.tensor_tensor(e_t[:, 0:w], ss["rz"][:, 1, 0:w],
                                        d_t[:, 0:w], A.mult)
                o_eng = nc.vector if ss["final"] else nc.gpsimd
                o_eng.tensor_tensor(ss["o"][:, ss["s0"]:ss["s0"] + w],
                                    n_t[:, 0:w], e_t[:, 0:w], A.add)
                if ss["g0"] + ss["W"] >= H1:
                    # last slab: per-sub-slab stores shorten the drain tail
                    nc.sync.dma_start(om[:, ss["g0"] + ss["s0"]:ss["g0"] + ss["s0"] + w],
                                      ss["o"][:, ss["s0"]:ss["s0"] + w])
                elif ss["last_in_slab"]:
                    nc.sync.dma_start(om[:, ss["g0"]:ss["g0"] + ss["W"]], ss["o"][:])

            # Stage C: transposes + psum evacuation (lag 3).
            def stage_c(ss):
                nt = ss["w"] // P
                tr_ps = trpool.tile([P, 4, 128], f16, tag="tr")
                nmT = gpool.tile([P, 4, 128], f16, tag="nmT")
                for k in range(nt):
                    nc.tensor.transpose(tr_ps[:, k, :],
                                        ss["o"][:, ss["s0"] + P * k:ss["s0"] + P * (k + 1)],
                                        id_t[:])
                nc.vector.tensor_copy(nmT[:, 0:nt, :], tr_ps[:, 0:nt, :])
                ss["nmT"] = nmT

            # Stage D: community matmuls (lag 4). The last node-tile carries
            # the accumulation-group stop flag (phase-0 matmuls all precede it
            # in PE program order).
            def stage_d(ss):
                for k in range(ss["w"] // P):
                    t_idx = (ss["g0"] + ss["s0"]) // P + k
                    nc.tensor.matmul(comm[:], ss["nmT"][:, k, :],
                                     ss["inc"][:, ss["s0"] // P + k, :],
                                     start=(t_idx == 0), stop=(t_idx == T1 - 1))

            subs = []
            emitted = {"b": 0, "c": 0, "d": 0}

            def pump(k):
                # Run lagged stages for cycle k of the pipeline.
                if k - 1 >= 0 and k - 1 < len(subs):
                    stage_b(subs[k - 1]); emitted["b"] = k
                if k - 3 >= 0 and k - 3 < len(subs):
                    stage_c(subs[k - 3]); emitted["c"] = k - 3
                if k - 5 >= 0 and k - 5 < len(subs):
                    stage_d(subs[k - 5]); emitted["d"] = k - 5
                    if h0_state["armed"]:
                        emit_h0(4)

            kk = 0
            pend_inc = []
            for g0, g1 in zip(slab_edges[:-1], slab_edges[1:]):
                W = g1 - g0
                x_s = xpool.tile([P, 4, W], f8, tag="x")
                nc.sync.dma_start(x_s[:], X8[:, :, g0:g0 + W])
                t8_s = xpool.tile([KD, W], f8, tag="t8")
                nc.gpsimd.dma_start(t8_s[:], t8[:, g0:g0 + W])
                mT_s = xpool.tile([P, W], f16, tag="mT")
                nc.scalar.dma_start(mT_s[:], mT[:, g0:g0 + W])
                # inc isn't consumed until the lag-4 comm stage: defer its DMA
                # one slab so early bandwidth goes to the gate streams.
                inc_s = xpool.tile([P, W // P, 256], f8e3, tag="inc")
                pend_inc.append((inc_s, g0, W))
                if len(pend_inc) > 1:
                    i_s, ig0, iW = pend_inc.pop(0)
                    nc.gpsimd.dma_start(i_s[:], inc1[:, ig0 // P:(ig0 + iW) // P, :])
                o_s = opool.tile([P, W], f16, tag="o")
                if g0 == 3072:
                    nc.scalar.dma_start(m0_t[:], m0[:])
                    nc.scalar.dma_start(inc0_t[:], inc0[:])
                    h0_state["dma"] = True
                elif g0 >= 5120:
                    h0_state["armed"] = h0_state["dma"]

                for s0 in range(0, W, SUB):
                    w = min(SUB, W - s0)
                    subs.append(dict(x=x_s, t8=t8_s, mT=mT_s, inc=inc_s, o=o_s,
                                     g0=g0, s0=s0, W=W, w=w,
                                     last_in_slab=(s0 + w >= W),
                                     final=(g0 + s0 + w >= H1)))
                    stage_a(subs[kk])
                    pump(kk)
                    kk += 1

            for i_s, ig0, iW in pend_inc:
                nc.gpsimd.dma_start(i_s[:], inc1[:, ig0 // P:(ig0 + iW) // P, :])

            if not h0_state["dma"]:  # few-slab edge case: load phase-0 now
                nc.scalar.dma_start(m0_t[:], m0[:])
                nc.scalar.dma_start(inc0_t[:], inc0[:])

            # Drain the pipeline; phase-0 remainder goes first so the final
            # community matmul (stop flag) is the true end of the psum group.
            emit_h0(T0)
            for k in range(kk, kk + 6):
                pump(k)

            cm = gpool.tile([P, 256], f32, tag="cm")
            nc.scalar.activation(cm[:], comm[:], AF.Copy)
            nc.sync.dma_start(ocm[:], cm[:])

    nc.compile()
    return nc


def _get_program(H1, H0):
    key = (H1, H0)
    if key not in _COMPILED:
        _COMPILED[key] = _build_program(H1, H0)
    return _COMPILED[key]


def _ceil_to(x, q):
    return (x + q - 1) // q * q


def kernel(src, dst, t, last_update, event_feat, src_embeds, dst_embeds,
           nodes_memory, incidence, w_time, b_time, W_ih, W_hh, b_ih, b_hh):
    src = np.asarray(src); dst = np.asarray(dst); t = np.asarray(t)
    last_update = np.asarray(last_update)
    event_feat = np.asarray(event_feat, np.float32)
    src_embeds = np.asarray(src_embeds, np.float32)
    dst_embeds = np.asarray(dst_embeds, np.float32)
    nodes_memory = np.asarray(nodes_memory, np.float32)
    incidence = np.asarray(incidence, np.float32)
    w_time = np.asarray(w_time, np.float32); b_time = np.asarray(b_time, np.float32)
    W_ih = np.asarray(W_ih, np.float32); W_hh = np.asarray(W_hh, np.float32)
    b_ih = np.asarray(b_ih, np.float32); b_hh = np.asarray(b_hh, np.float32)

    # ---- Host routing: 'last' aggregation = stable-sort scatter (index-only) ----
    src_all = np.concatenate([src, dst])
    t_all = np.concatenate([t, t])
    perm = np.argsort(t_all, kind="stable")
    win = np.zeros(N, np.int64)
    win[src_all[perm]] = perm          # ascending rank; last write = newest event
    has = np.bincount(src_all, minlength=N) > 0

    nodes1 = np.where(has)[0]
    nodes0 = np.where(~has)[0]
    n1, n0 = len(nodes1), len(nodes0)
    h1c = (n1 + NCORES - 1) // NCORES
    h0c = (n0 + NCORES - 1) // NCORES
    H1 = _ceil_to(max(h1c, 1), 256)
    H0 = _ceil_to(max(h0c, 1), 256)
    T0 = H0 // P

    # Winner-event data for has-event nodes
    w1 = win[nodes1]
    dtw = (t_all[w1] - last_update[nodes1]).astype(np.float32)
    tenc = np.cos(dtw[:, None] * w_time[None, :] + b_time[None, :])  # [n1, T]
    lt = w1 < E
    w0 = np.where(lt, w1, w1 - E)
    emb_s = np.where(lt[:, None], src_embeds[w0], dst_embeds[w0])
    emb_d = np.where(lt[:, None], dst_embeds[w0], src_embeds[w0])
    feat = event_feat[w0]

    # ---- Replicated weights: fp8 DoubleRow pairs (W8, Wlo), pre-scaled by 8 ----
    bias = (b_ih + b_hh).astype(np.float32)
    chunks = {}  # name -> [128, 384] f32 weight rows
    chunks['A'] = W_ih[0:128]
    chunks['B'] = W_ih[128:256]
    chunks['C'] = W_ih[256:384]
    Dr = np.zeros((128, 384), np.float32)
    Dr[0:T] = W_ih[384:448]
    Dr[T + 1] = bias                      # bias lane (stream row = 1.0)
    chunks['D'] = Dr
    chunks['E'] = W_hh
    # Slot layout: r (slots 0-2, no residual): [A8|B8], [E8|C8], [D8|0];
    # z (3-7), xn (8-11), hn (12): (W8, Wlo) residual pairs per chunk.
    WPa = np.zeros((P, 13, 2, 128), np.float32)

    def _hi(cname, seg):
        wseg = chunks[cname][:, 128 * seg:128 * (seg + 1)] * WS
        if seg == 1 and cname == 'D':   # z-seg D chunk carries the has-mask row
            wseg = wseg.copy()
            wseg[T] = ZBIG
        return wseg, _q8(wseg).astype(np.float32)

    WPa[:, 0, 0, :] = _hi('A', 0)[1]
    WPa[:, 0, 1, :] = _hi('B', 0)[1]
    WPa[:, 1, 0, :] = _hi('E', 0)[1]
    WPa[:, 1, 1, :] = _hi('C', 0)[1]
    WPa[:, 2, 0, :] = _hi('D', 0)[1]
    for j, (seg, cname) in enumerate([(1, c) for c in "ABCDE"] +
                                     [(2, c) for c in "ABCD"] + [(2, 'E')], start=3):
        wseg, hi = _hi(cname, seg)
        WPa[:, j, 0, :] = hi
        WPa[:, j, 1, :] = _q8(wseg - hi).astype(np.float32)
    WP_v = WPa.astype(NP_E4)

    ident = np.eye(P, dtype=np.float16)

    nc = _get_program(H1, H0)

    in_maps = []
    core_n1 = []
    for c in range(NCORES):
        i0, i1 = c * h1c, min((c + 1) * h1c, n1)
        cn1 = max(i1 - i0, 0)
        core_n1.append((i0, i1))
        sl = slice(i0, i1)
        X8c = np.zeros((P, 4, H1), NP_E4)
        X8c[:, 0, :cn1] = _q8(emb_s[sl].T)
        X8c[:, 1, :cn1] = _q8(emb_d[sl].T)
        X8c[:, 2, :cn1] = _q8(nodes_memory[nodes1[sl]].T)
        X8c[:, 3, :cn1] = _q8(feat[sl].T)
        tpl = np.zeros((T + 2, H1), np.float32)
        tpl[0:T, :cn1] = tenc[sl].T
        tpl[T, cn1:] = 1.0                # mask row: 1 on padding columns
        tpl[T + 1, :] = 1.0               # bias lane
        t8c = _q8(tpl)
        mTc = np.zeros((P, H1), np.float16)
        mTc[:, :cn1] = nodes_memory[nodes1[sl]].T.astype(np.float16)
        inc1c = np.zeros((H1 // P, P, 256), NP_E3)
        inc1c.reshape(H1, 256)[:cn1] = _q8e3(incidence[nodes1[sl]])
        inc1c = np.ascontiguousarray(inc1c.transpose(1, 0, 2))

        j0, j1 = c * h0c, min((c + 1) * h0c, n0)
        cn0 = max(j1 - j0, 0)
        sl0 = nodes0[j0:j1]
        m0c = np.zeros((H0, 128), NP_E3)
        m0c[:cn0] = _q8e3(nodes_memory[sl0])
        m0c = np.ascontiguousarray(m0c.reshape(T0, P, 128).transpose(1, 0, 2))
        inc0c = np.zeros((H0, 256), NP_E3)
        inc0c[:cn0] = _q8e3(incidence[sl0])
        inc0c = np.ascontiguousarray(inc0c.reshape(T0, P, 256).transpose(1, 0, 2))

        in_maps.append(dict(X8=X8c, t8=t8c, mT=mTc, inc1=inc1c, m0=m0c, inc0=inc0c,
                            WP=WP_v, idt=ident))

    res = run_bass_kernel_spmd(nc, in_maps, core_ids=list(range(NCORES)))

    out = np.empty((N + C, M), np.float32)
    out[:N] = nodes_memory
    comm = np.zeros((M, C), np.float64)
    for c in range(NCORES):
        i0, i1 = core_n1[c]
        if i1 > i0:
            out[nodes1[i0:i1]] = res.results[c]["om"][:, :i1 - i0].T.astype(np.float32)
        comm += res.results[c]["ocm"]
    out[N:] = comm.T.astype(np.float32)
    return out


# revision 63
# speedup vs baseline: 1.0078x; 1.0065x over previous
"""Distributed Trainium2 kernel for nn_CONNECT_86964497809993 (TGN-style
GNN message passing: last-event aggregation + GRU memory update + community
incidence matmul), sharded over 8 NeuronCores.

Strategy: event routing ("last message per node") is integer index plumbing
done on the host during input sharding (per the sharding hint); nodes are
then re-partitioned across cores into a has-event set (full GRU pipeline)
and a no-event set (memory passthrough: only the community matmul needs
those rows). Per core:
  - gates   psum = [embs|embd|feat|tenc+mask|mem] @ [W_ih;W_hh]*8 computed
            with fp8e4 DoubleRow matmuls (2 K-tiles per instr, 0.5 cyc/row).
            Each 128-row K-chunk is a DR pair (W8, Wlo) sharing one
            stride-0-broadcast fp8 stream, where W8 = fp8(8W) and
            Wlo = fp8(8W - W8): the fp8 weight-residual kills the
            systematic per-column quantization bias that otherwise
            accumulates over the 100k-node community reduction.
  - mask    has-mask folded into the z-gate as an extra stream row with
            weight 240 (sigma(30) = 1 => passthrough), zero elementwise cost.
  - GRU     ACT: sigmoid/tanh at scale 1/8 (un-scales the 8x fp8 weights);
            DVE: r*hn, xn+ (psum-coupled), two blend ops + psum evacuation;
            Pool: the final blend add (off the critical chain, lag-3 tail).
  - comm    new_mem tiles PE-transposed (f16, via identity) into PSUM,
            evacuated by DVE tensor_copy, then f16-stationary x fp8e3-moving
            matmuls accumulate incidence^T partials; no-event nodes
            contribute via fp8e3 matmuls directly from memory tiles, spread
            through the pipeline to fill PE gaps. incidence/memory use
            float8e3 (e3m4: 4 mantissa bits) - their error enters a 100k-term
            reduction, and e4m3 would double it.
All streams are feature-major [feat, node] so every DMA moves >=1KB
contiguous runs per partition (full DMA bus rate) and memory is loaded
once. The whole emission is software-pipelined 6 deep across the five
in-order engine queues (gates/sigma -> tanh/blend -> transpose/evac ->
comm) with PE p-state warm-up at t=0 and progressive DMA slab sizes.
Community partials ([128,256] per core) are summed on the host."""

import numpy as np
import ml_dtypes

from concourse import bacc
import concourse.mybir as mybir
from concourse.tile import TileContext
from concourse.bass_utils import run_bass_kernel_spmd

N, E, C = 100000, 50000, 256
M, D, F, T = 128, 128, 128, 64
NCORES = 8
P = 128

f32 = mybir.dt.float32
f16 = mybir.dt.float16
f8 = mybir.dt.float8e4
f8e3 = mybir.dt.float8e3
A = mybir.AluOpType
AF = mybir.ActivationFunctionType
DR = mybir.MatmulPerfMode.DoubleRow

NP_E4 = ml_dtypes.float8_e4m3
NP_E3 = ml_dtypes.float8_e3m4
WS = 8.0                  # weight pre-scale (un-done by ACT scale=1/8)
ZBIG = 240.0              # mask weight: sigma(240/8) = 1.0
DMA_SLAB = 1024           # has1 DMA slab (columns)
SUB = 512                 # compute sub-slab (columns)

_COMPILED = {}            # (H1, H0) -> compiled program


def _q8(a):
    return np.asarray(a, np.float32).astype(NP_E4)


def _q8e3(a):
    return np.asarray(a, np.float32).astype(NP_E3)


def _build_program(H1, H0):
    T1 = H1 // P
    T0 = H0 // P
    KD = T + 2  # D-chunk contraction depth: 64 tenc + mask + bias
    nc = bacc.Bacc("TRN2", target_bir_lowering=False)

    X8 = nc.dram_tensor("X8", [P, 4, H1], f8, kind="ExternalInput")
    t8 = nc.dram_tensor("t8", [KD, H1], f8, kind="ExternalInput")
    mT = nc.dram_tensor("mT", [P, H1], f16, kind="ExternalInput")
    inc1 = nc.dram_tensor("inc1", [P, T1, 256], f8e3, kind="ExternalInput")
    m0 = nc.dram_tensor("m0", [P, T0, 128], f8e3, kind="ExternalInput")
    inc0 = nc.dram_tensor("inc0", [P, T0, 256], f8e3, kind="ExternalInput")
    WP = nc.dram_tensor("WP", [P, 13, 2, 128], f8, kind="ExternalInput")
    idt = nc.dram_tensor("idt", [P, P], f16, kind="ExternalInput")
    om = nc.dram_tensor("om", [P, H1], f16, kind="ExternalOutput")
    ocm = nc.dram_tensor("ocm", [P, 256], f32, kind="ExternalOutput")

    # X8 planes: 0=emb_s 1=emb_d 2=mem 3=feat; plane 4 = t8 (66-deep chunk).
    # r-gate runs without the weight-residual (numerically validated): its
    # k-tile pairs are adjacent X8 plane pairs (s,d) and (m,f) plus t8, so it
    # needs only 3 DoubleRow instructions. z/xn/hn keep (W8,Wlo) pairs on a
    # stride-0-duplicated stream.
    SEGS = [
        (1, [3, 4, 5, 6, 7], [0, 1, 3, 4, 2]),   # z   <- psum rz[:,1,:]
        (2, [8, 9, 10, 11], [0, 1, 3, 4]),       # xn  <- psum xn[:,0,:]
        (3, [12], [2]),                          # hn  <- psum xn[:,1,:]
    ]

    with TileContext(nc) as tc:
        with tc.tile_pool(name="const", bufs=1) as cpool, \
             tc.tile_pool(name="xp", bufs=3) as xpool, \
             tc.tile_pool(name="gp", bufs=3) as gpool, \
             tc.tile_pool(name="op", bufs=3) as opool, \
             tc.tile_pool(name="rzp", bufs=1, space="PSUM") as rzpool, \
             tc.tile_pool(name="xnp", bufs=2, space="PSUM") as xnpool, \
             tc.tile_pool(name="trp", bufs=1, space="PSUM") as trpool, \
             tc.tile_pool(name="cap", bufs=1, space="PSUM") as capool:

            wp_t = cpool.tile([P, 13, 2, 128], f8)
            nc.sync.dma_start(wp_t[:, 0:3], WP[:, 0:3])   # r-gate slots first
            nc.scalar.dma_start(wp_t[:, 3:13], WP[:, 3:13])
            id_t = cpool.tile([P, P], f16)
            nc.scalar.dma_start(id_t[:], idt[:])
            comm = capool.tile([P, 256], f32)
            m0_t = cpool.tile([P, T0, 128], f8e3)
            inc0_t = cpool.tile([P, T0, 256], f8e3)

            # PE p-state warm-up: dependency-free matmuls on a zeroed scratch
            # tile keep the tensor engine continuously busy through the first
            # input DMAs so real gate matmuls start at full clock. Results land
            # in the comm psum bank, which the first real community matmul
            # resets via its start flag.
            warm = gpool.tile([P, 512], f16, tag="warm")
            nc.gpsimd.memset(warm[:], 0.0)
            for _ in range(20):
                nc.tensor.matmul(comm[:], warm[:, 0:128], warm[:, 256:512],
                                 start=True, stop=True, skip_group_check=True)

            # ---- Phase 1: has-event nodes ----
            # Fully software-pipelined across in-order engine queues:
            #   cycle k emits: gates(k) [PE], sigma(k) [ACT], tt/npre(k) [DVE],
            #   then the LAGGED stages: tanh(k-1) [ACT] (so sigma(k) is never
            #   queued behind a tanh that waits on DVE), blend(k-1) [Pool+DVE],
            #   transposes(k-3) [PE], evac(k-3) [DVE], comm(k-4) [PE], and a
            #   few phase-0 matmuls to fill PE gaps.
            h0_state = {"next": 0, "armed": False, "dma": False}

            def emit_h0(count):
                p = h0_state["next"]
                while count > 0 and p < T0:
                    nc.tensor.matmul(comm[:], m0_t[:, p, :], inc0_t[:, p, :],
                                     start=False, stop=False)
                    p += 1; count -= 1
                h0_state["next"] = p

            # Progressive slab sizes: small first slabs start PE early; later,
            # bigger transfers hide behind compute.
            slab_edges = [0]
            for w in (512, 512, 1024, 1024):
                if slab_edges[-1] + w < H1:
                    slab_edges.append(slab_edges[-1] + w)
            while slab_edges[-1] < H1:
                slab_edges.append(min(slab_edges[-1] + DMA_SLAB, H1))

            # Stage A: gate matmuls + sigma + tt/npre for sub-slab ss.
            def stage_a(ss):
                x_s, t8_s, mT_s, inc_s, o_s, g0, s0, w = (
                    ss["x"], ss["t8"], ss["mT"], ss["inc"], ss["o"],
                    ss["g0"], ss["s0"], ss["w"])
                rz_ps = rzpool.tile([P, 2, SUB], f32, tag="rz")
                xn_ps = xnpool.tile([P, 2, SUB], f32, tag="xn")
                halves = []
                for h0c in range(0, w, 256):
                    cs = slice(s0 + h0c, s0 + h0c + 256)
                    rhs = [x_s[:, pl, cs].unsqueeze(1).broadcast_to([P, 2, 256])
                           for pl in range(4)]
                    rhs.append(t8_s[:, cs].unsqueeze(1).broadcast_to([KD, 2, 256]))
                    halves.append((h0c, cs, rhs))
                # r/z for BOTH halves first so sigma's input is ready early;
                # r-gate: (s,d) and (m,f) adjacent-plane pairs + t8.
                for h0c, cs, rhs in halves:
                    dest = rz_ps[:, 0, h0c:h0c + 256]
                    nc.tensor.matmul(dest, wp_t[:, 0, :, :], x_s[:, 0:2, cs],
                                     start=True, stop=False, perf_mode=DR)
                    nc.tensor.matmul(dest, wp_t[:, 1, :, :], x_s[:, 2:4, cs],
                                     start=False, stop=False, perf_mode=DR)
                    nc.tensor.matmul(dest, wp_t[0:KD, 2, :, :], rhs[4],
                                     start=False, stop=True, perf_mode=DR)
                    d_idx, pairs, planes = SEGS[0]
                    dest = rz_ps[:, 1, h0c:h0c + 256]
                    for i, (j, pl) in enumerate(zip(pairs, planes)):
                        lhs = wp_t[0:KD, j, :, :] if pl == 4 else wp_t[:, j, :, :]
                        nc.tensor.matmul(dest, lhs, rhs[pl],
                                         start=(i == 0), stop=(i == len(pairs) - 1),
                                         perf_mode=DR)
                for h0c, cs, rhs in halves:
                    for d_idx, pairs, planes in SEGS[1:]:
                        dest = xn_ps[:, d_idx % 2, h0c:h0c + 256]
                        for i, (j, pl) in enumerate(zip(pairs, planes)):
                            lhs = wp_t[0:KD, j, :, :] if pl == 4 else wp_t[:, j, :, :]
                            nc.tensor.matmul(dest, lhs, rhs[pl],
                                             start=(i == 0), stop=(i == len(pairs) - 1),
                                             perf_mode=DR)
                rz = gpool.tile([P, 2, SUB], f16, tag="rzs")
                nc.scalar.activation(rz[:, :, 0:w], rz_ps[:, :, 0:w],
                                     AF.Sigmoid, scale=1.0 / WS)
                tt = gpool.tile([P, SUB], f16, tag="tt")
                nc.vector.tensor_tensor(tt[:, 0:w], rz[:, 0, 0:w],
                                        xn_ps[:, 1, 0:w], A.mult)
                npre = gpool.tile([P, SUB], f16, tag="npre")
                nc.vector.tensor_tensor(npre[:, 0:w], xn_ps[:, 0, 0:w],
                                        tt[:, 0:w], A.add)
                ss["rz"], ss["npre"] = rz, npre

            # Stage B: tanh + blend for sub-slab ss (one cycle after stage A).
            # The final sub-slab keeps its whole blend on DVE: the Pool hop
            # would add ~800ns to the drain's serial dependency chain.
            def stage_b(ss):
                w = ss["w"]
                n_t = gpool.tile([P, SUB], f16, tag="n")
                nc.scalar.activation(n_t[:, 0:w], ss["npre"][:, 0:w],
                                     AF.Tanh, scale=1.0 / WS)
                d_t = gpool.tile([P, SUB], f16, tag="d")
                nc.vector.tensor_tensor(d_t[:, 0:w],
                                        ss["mT"][:, ss["s0"]:ss["s0"] + w],
                                        n_t[:, 0:w], A.subtract)
                e_t = gpool.tile([P, SUB], f16, tag="e")
                nc.vector.tensor_tensor(e_t[:, 0:w], ss["rz"][:, 1, 0:w],
                                        d_t[:, 0:w], A.mult)
                o_eng = nc.vector if ss["final"] else nc.gpsimd
                o_eng.tensor_tensor(ss["o"][:, ss["s0"]:ss["s0"] + w],
                                    n_t[:, 0:w], e_t[:, 0:w], A.add)
                if ss["g0"] + ss["W"] >= H1:
                    # last slab: per-sub-slab stores via Pool/SWDGE (skips the
                    # HWDGE+DGE latency sitting on the drain's critical tail)
                    nc.gpsimd.dma_start(om[:, ss["g0"] + ss["s0"]:ss["g0"] + ss["s0"] + w],
                                        ss["o"][:, ss["s0"]:ss["s0"] + w])
                elif ss["last_in_slab"]:
                    nc.sync.dma_start(om[:, ss["g0"]:ss["g0"] + ss["W"]], ss["o"][:])

            # Stage C: transposes + psum evacuation (lag 3).
            def stage_c(ss):
                nt = ss["w"] // P
                tr_ps = trpool.tile([P, 4, 128], f16, tag="tr")
                nmT = gpool.tile([P, 4, 128], f16, tag="nmT")
                for k in range(nt):
                    nc.tensor.transpose(tr_ps[:, k, :],
                                        ss["o"][:, ss["s0"] + P * k:ss["s0"] + P * (k + 1)],
                                        id_t[:])
                nc.vector.tensor_copy(nmT[:, 0:nt, :], tr_ps[:, 0:nt, :])
                ss["nmT"] = nmT

            # Stage D: community matmuls (lag 4). The last node-tile carries
            # the accumulation-group stop flag (phase-0 matmuls all precede it
            # in PE program order).
            def stage_d(ss):
                for k in range(ss["w"] // P):
                    t_idx = (ss["g0"] + ss["s0"]) // P + k
                    nc.tensor.matmul(comm[:], ss["nmT"][:, k, :],
                                     ss["inc"][:, ss["s0"] // P + k, :],
                                     start=(t_idx == 0), stop=(t_idx == T1 - 1))

            subs = []
            emitted = {"b": 0, "c": 0, "d": 0}

            def pump(k):
                # Run lagged stages for cycle k of the pipeline.
                if k - 1 >= 0 and k - 1 < len(subs):
                    stage_b(subs[k - 1]); emitted["b"] = k
                if k - 3 >= 0 and k - 3 < len(subs):
                    stage_c(subs[k - 3]); emitted["c"] = k - 3
                if k - 5 >= 0 and k - 5 < len(subs):
                    stage_d(subs[k - 5]); emitted["d"] = k - 5
                    if h0_state["armed"]:
                        emit_h0(4)

            kk = 0
            pend_inc = []
            for g0, g1 in zip(slab_edges[:-1], slab_edges[1:]):
                W = g1 - g0
                x_s = xpool.tile([P, 4, W], f8, tag="x")
                nc.sync.dma_start(x_s[:], X8[:, :, g0:g0 + W])
                t8_s = xpool.tile([KD, W], f8, tag="t8")
                nc.gpsimd.dma_start(t8_s[:], t8[:, g0:g0 + W])
                mT_s = xpool.tile([P, W], f16, tag="mT")
                nc.scalar.dma_start(mT_s[:], mT[:, g0:g0 + W])
                # inc isn't consumed until the lag-4 comm stage: defer its DMA
                # one slab so early bandwidth goes to the gate streams.
                inc_s = xpool.tile([P, W // P, 256], f8e3, tag="inc")
                pend_inc.append((inc_s, g0, W))
                if len(pend_inc) > 1:
                    i_s, ig0, iW = pend_inc.pop(0)
                    nc.gpsimd.dma_start(i_s[:], inc1[:, ig0 // P:(ig0 + iW) // P, :])
                o_s = opool.tile([P, W], f16, tag="o")
                if g0 == 3072:
                    nc.scalar.dma_start(m0_t[:], m0[:])
                    nc.scalar.dma_start(inc0_t[:], inc0[:])
                    h0_state["dma"] = True
                elif g0 >= 5120:
                    h0_state["armed"] = h0_state["dma"]

                for s0 in range(0, W, SUB):
                    w = min(SUB, W - s0)
                    subs.append(dict(x=x_s, t8=t8_s, mT=mT_s, inc=inc_s, o=o_s,
                                     g0=g0, s0=s0, W=W, w=w,
                                     last_in_slab=(s0 + w >= W),
                                     final=(g0 + s0 + w >= H1)))
                    stage_a(subs[kk])
                    pump(kk)
                    kk += 1

            for i_s, ig0, iW in pend_inc:
                nc.gpsimd.dma_start(i_s[:], inc1[:, ig0 // P:(ig0 + iW) // P, :])

            if not h0_state["dma"]:  # few-slab edge case: load phase-0 now
                nc.scalar.dma_start(m0_t[:], m0[:])
                nc.scalar.dma_start(inc0_t[:], inc0[:])

            # Drain the pipeline; phase-0 remainder goes first so the final
            # community matmul (stop flag) is the true end of the psum group.
            emit_h0(T0)
            for k in range(kk, kk + 6):
                pump(k)

            cm = gpool.tile([P, 256], f32, tag="cm")
            nc.vector.tensor_copy(cm[:], comm[:])
            nc.sync.dma_start(ocm[:], cm[:])

    nc.compile()
    return nc


def _get_program(H1, H0):
    key = (H1, H0)
    if key not in _COMPILED:
        _COMPILED[key] = _build_program(H1, H0)
    return _COMPILED[key]


def _ceil_to(x, q):
    return (x + q - 1) // q * q


def kernel(src, dst, t, last_update, event_feat, src_embeds, dst_embeds,
           nodes_memory, incidence, w_time, b_time, W_ih, W_hh, b_ih, b_hh):
    src = np.asarray(src); dst = np.asarray(dst); t = np.asarray(t)
    last_update = np.asarray(last_update)
    event_feat = np.asarray(event_feat, np.float32)
    src_embeds = np.asarray(src_embeds, np.float32)
    dst_embeds = np.asarray(dst_embeds, np.float32)
    nodes_memory = np.asarray(nodes_memory, np.float32)
    incidence = np.asarray(incidence, np.float32)
    w_time = np.asarray(w_time, np.float32); b_time = np.asarray(b_time, np.float32)
    W_ih = np.asarray(W_ih, np.float32); W_hh = np.asarray(W_hh, np.float32)
    b_ih = np.asarray(b_ih, np.float32); b_hh = np.asarray(b_hh, np.float32)

    # ---- Host routing: 'last' aggregation = stable-sort scatter (index-only) ----
    src_all = np.concatenate([src, dst])
    t_all = np.concatenate([t, t])
    perm = np.argsort(t_all, kind="stable")
    win = np.zeros(N, np.int64)
    win[src_all[perm]] = perm          # ascending rank; last write = newest event
    has = np.bincount(src_all, minlength=N) > 0

    nodes1 = np.where(has)[0]
    nodes0 = np.where(~has)[0]
    n1, n0 = len(nodes1), len(nodes0)
    h1c = (n1 + NCORES - 1) // NCORES
    h0c = (n0 + NCORES - 1) // NCORES
    H1 = _ceil_to(max(h1c, 1), 256)
    H0 = _ceil_to(max(h0c, 1), 256)
    T0 = H0 // P

    # Winner-event data for has-event nodes
    w1 = win[nodes1]
    dtw = (t_all[w1] - last_update[nodes1]).astype(np.float32)
    tenc = np.cos(dtw[:, None] * w_time[None, :] + b_time[None, :])  # [n1, T]
    lt = w1 < E
    w0 = np.where(lt, w1, w1 - E)
    emb_s = np.where(lt[:, None], src_embeds[w0], dst_embeds[w0])
    emb_d = np.where(lt[:, None], dst_embeds[w0], src_embeds[w0])
    feat = event_feat[w0]

    # ---- Replicated weights: fp8 DoubleRow pairs (W8, Wlo), pre-scaled by 8 ----
    bias = (b_ih + b_hh).astype(np.float32)
    chunks = {}  # name -> [128, 384] f32 weight rows
    chunks['A'] = W_ih[0:128]
    chunks['B'] = W_ih[128:256]
    chunks['C'] = W_ih[256:384]
    Dr = np.zeros((128, 384), np.float32)
    Dr[0:T] = W_ih[384:448]
    Dr[T + 1] = bias                      # bias lane (stream row = 1.0)
    chunks['D'] = Dr
    chunks['E'] = W_hh
    # Slot layout: r (slots 0-2, no residual): [A8|B8], [E8|C8], [D8|0];
    # z (3-7), xn (8-11), hn (12): (W8, Wlo) residual pairs per chunk.
    WPa = np.zeros((P, 13, 2, 128), np.float32)

    def _hi(cname, seg):
        wseg = chunks[cname][:, 128 * seg:128 * (seg + 1)] * WS
        if seg == 1 and cname == 'D':   # z-seg D chunk carries the has-mask row
            wseg = wseg.copy()
            wseg[T] = ZBIG
        return wseg, _q8(wseg).astype(np.float32)

    WPa[:, 0, 0, :] = _hi('A', 0)[1]
    WPa[:, 0, 1, :] = _hi('B', 0)[1]
    WPa[:, 1, 0, :] = _hi('E', 0)[1]
    WPa[:, 1, 1, :] = _hi('C', 0)[1]
    WPa[:, 2, 0, :] = _hi('D', 0)[1]
    for j, (seg, cname) in enumerate([(1, c) for c in "ABCDE"] +
                                     [(2, c) for c in "ABCD"] + [(2, 'E')], start=3):
        wseg, hi = _hi(cname, seg)
        WPa[:, j, 0, :] = hi
        WPa[:, j, 1, :] = _q8(wseg - hi).astype(np.float32)
    WP_v = WPa.astype(NP_E4)

    ident = np.eye(P, dtype=np.float16)

    nc = _get_program(H1, H0)

    in_maps = []
    core_n1 = []
    for c in range(NCORES):
        i0, i1 = c * h1c, min((c + 1) * h1c, n1)
        cn1 = max(i1 - i0, 0)
        core_n1.append((i0, i1))
        sl = slice(i0, i1)
        X8c = np.zeros((P, 4, H1), NP_E4)
        X8c[:, 0, :cn1] = _q8(emb_s[sl].T)
        X8c[:, 1, :cn1] = _q8(emb_d[sl].T)
        X8c[:, 2, :cn1] = _q8(nodes_memory[nodes1[sl]].T)
        X8c[:, 3, :cn1] = _q8(feat[sl].T)
        tpl = np.zeros((T + 2, H1), np.float32)
        tpl[0:T, :cn1] = tenc[sl].T
        tpl[T, cn1:] = 1.0                # mask row: 1 on padding columns
        tpl[T + 1, :] = 1.0               # bias lane
        t8c = _q8(tpl)
        mTc = np.zeros((P, H1), np.float16)
        mTc[:, :cn1] = nodes_memory[nodes1[sl]].T.astype(np.float16)
        inc1c = np.zeros((H1 // P, P, 256), NP_E3)
        inc1c.reshape(H1, 256)[:cn1] = _q8e3(incidence[nodes1[sl]])
        inc1c = np.ascontiguousarray(inc1c.transpose(1, 0, 2))

        j0, j1 = c * h0c, min((c + 1) * h0c, n0)
        cn0 = max(j1 - j0, 0)
        sl0 = nodes0[j0:j1]
        m0c = np.zeros((H0, 128), NP_E3)
        m0c[:cn0] = _q8e3(nodes_memory[sl0])
        m0c = np.ascontiguousarray(m0c.reshape(T0, P, 128).transpose(1, 0, 2))
        inc0c = np.zeros((H0, 256), NP_E3)
        inc0c[:cn0] = _q8e3(incidence[sl0])
        inc0c = np.ascontiguousarray(inc0c.reshape(T0, P, 256).transpose(1, 0, 2))

        in_maps.append(dict(X8=X8c, t8=t8c, mT=mTc, inc1=inc1c, m0=m0c, inc0=inc0c,
                            WP=WP_v, idt=ident))

    res = run_bass_kernel_spmd(nc, in_maps, core_ids=list(range(NCORES)))

    out = np.empty((N + C, M), np.float32)
    out[:N] = nodes_memory
    comm = np.zeros((M, C), np.float64)
    for c in range(NCORES):
        i0, i1 = core_n1[c]
        if i1 > i0:
            out[nodes1[i0:i1]] = res.results[c]["om"][:, :i1 - i0].T.astype(np.float32)
        comm += res.results[c]["ocm"]
    out[N:] = comm.T.astype(np.float32)
    return out


# revision 65
# speedup vs baseline: 1.0650x; 1.0567x over previous
"""Distributed Trainium2 kernel for nn_CONNECT_86964497809993 (TGN-style
GNN message passing: last-event aggregation + GRU memory update + community
incidence matmul), sharded over 8 NeuronCores.

Strategy: event routing ("last message per node") is integer index plumbing
done on the host during input sharding (per the sharding hint); nodes are
then re-partitioned across cores into a has-event set (full GRU pipeline)
and a no-event set (memory passthrough: only the community matmul needs
those rows). Per core:
  - gates   psum = [embs|embd|feat|tenc+mask|mem] @ [W_ih;W_hh]*8 computed
            with fp8e4 DoubleRow matmuls (2 K-tiles per instr, 0.5 cyc/row).
            Each 128-row K-chunk is a DR pair (W8, Wlo) sharing one
            stride-0-broadcast fp8 stream, where W8 = fp8(8W) and
            Wlo = fp8(8W - W8): the fp8 weight-residual kills the
            systematic per-column quantization bias that otherwise
            accumulates over the 100k-node community reduction.
  - mask    has-mask folded into the z-gate as an extra stream row with
            weight 240 (sigma(30) = 1 => passthrough), zero elementwise cost.
  - GRU     ACT: sigmoid/tanh at scale 1/8 (un-scales the 8x fp8 weights);
            DVE: r*hn, xn+ (psum-coupled), two blend ops + psum evacuation;
            Pool: the final blend add (off the critical chain, lag-3 tail).
  - comm    new_mem tiles PE-transposed (f16, via identity) into PSUM,
            evacuated by DVE tensor_copy, then f16-stationary x fp8e3-moving
            matmuls accumulate incidence^T partials; no-event nodes
            contribute via fp8e3 matmuls directly from memory tiles, spread
            through the pipeline to fill PE gaps. incidence/memory use
            float8e3 (e3m4: 4 mantissa bits) - their error enters a 100k-term
            reduction, and e4m3 would double it.
All streams are feature-major [feat, node] so every DMA moves >=1KB
contiguous runs per partition (full DMA bus rate) and memory is loaded
once. The whole emission is software-pipelined 6 deep across the five
in-order engine queues (gates/sigma -> tanh/blend -> transpose/evac ->
comm) with PE p-state warm-up at t=0 and progressive DMA slab sizes.
Community partials ([128,256] per core) are summed on the host."""

import numpy as np
import ml_dtypes

from concourse import bacc
import concourse.mybir as mybir
from concourse.tile import TileContext
from concourse.bass_utils import run_bass_kernel_spmd

N, E, C = 100000, 50000, 256
M, D, F, T = 128, 128, 128, 64
NCORES = 8
P = 128

f32 = mybir.dt.float32
f16 = mybir.dt.float16
f8 = mybir.dt.float8e4
f8e3 = mybir.dt.float8e3
A = mybir.AluOpType
AF = mybir.ActivationFunctionType
DR = mybir.MatmulPerfMode.DoubleRow

NP_E4 = ml_dtypes.float8_e4m3
NP_E3 = ml_dtypes.float8_e3m4
WS = 8.0                  # weight pre-scale (un-done by ACT scale=1/8)
ZBIG = 240.0              # mask weight: sigma(240/8) = 1.0
DMA_SLAB = 1024           # has1 DMA slab (columns)
SUB = 512                 # compute sub-slab (columns)

_COMPILED = {}            # (H1, H0) -> compiled program


def _q8(a):
    return np.asarray(a, np.float32).astype(NP_E4)


def _q8e3(a):
    return np.asarray(a, np.float32).astype(NP_E3)


def _build_program(H1, H0):
    T1 = H1 // P
    T0 = H0 // P
    KD = T + 2  # D-chunk contraction depth: 64 tenc + mask + bias
    nc = bacc.Bacc("TRN2", target_bir_lowering=False)

    X8 = nc.dram_tensor("X8", [P, 4, H1], f8, kind="ExternalInput")
    t8 = nc.dram_tensor("t8", [KD, H1], f8, kind="ExternalInput")
    mT = nc.dram_tensor("mT", [P, H1], f16, kind="ExternalInput")
    inc1 = nc.dram_tensor("inc1", [P, T1, 256], f8e3, kind="ExternalInput")
    m0 = nc.dram_tensor("m0", [P, T0, 128], f8e3, kind="ExternalInput")
    inc0 = nc.dram_tensor("inc0", [P, T0, 256], f8e3, kind="ExternalInput")
    WP = nc.dram_tensor("WP", [P, 13, 2, 128], f8, kind="ExternalInput")
    idt = nc.dram_tensor("idt", [P, P], f16, kind="ExternalInput")
    om = nc.dram_tensor("om", [P, H1], f16, kind="ExternalOutput")
    ocm = nc.dram_tensor("ocm", [P, 256], f32, kind="ExternalOutput")

    # X8 planes: 0=emb_s 1=emb_d 2=mem 3=feat; plane 4 = t8 (66-deep chunk).
    # r-gate runs without the weight-residual (numerically validated): its
    # k-tile pairs are adjacent X8 plane pairs (s,d) and (m,f) plus t8, so it
    # needs only 3 DoubleRow instructions. z/xn/hn keep (W8,Wlo) pairs on a
    # stride-0-duplicated stream.
    SEGS = [
        (1, [3, 4, 5, 6, 7], [0, 1, 3, 4, 2]),   # z   <- psum rz[:,1,:]
        (2, [8, 9, 10, 11], [0, 1, 3, 4]),       # xn  <- psum xn[:,0,:]
        (3, [12], [2]),                          # hn  <- psum xn[:,1,:]
    ]

    with TileContext(nc) as tc:
        with tc.tile_pool(name="const", bufs=1) as cpool, \
             tc.tile_pool(name="xp", bufs=3) as xpool, \
             tc.tile_pool(name="gp", bufs=3) as gpool, \
             tc.tile_pool(name="op", bufs=3) as opool, \
             tc.tile_pool(name="rp", bufs=1, space="PSUM") as rpool, \
             tc.tile_pool(name="zp", bufs=1, space="PSUM") as zpool, \
             tc.tile_pool(name="xnp", bufs=2, space="PSUM") as xnpool, \
             tc.tile_pool(name="hnp", bufs=2, space="PSUM") as hnpool, \
             tc.tile_pool(name="trp", bufs=1, space="PSUM") as trpool, \
             tc.tile_pool(name="cap", bufs=1, space="PSUM") as capool:

            wp_t = cpool.tile([P, 13, 2, 128], f8)
            nc.sync.dma_start(wp_t[:, 0:3], WP[:, 0:3])   # r-gate slots first
            nc.scalar.dma_start(wp_t[:, 3:13], WP[:, 3:13])
            id_t = cpool.tile([P, P], f16)
            nc.scalar.dma_start(id_t[:], idt[:])
            comm = capool.tile([P, 256], f32)
            m0_t = cpool.tile([P, T0, 128], f8e3)
            inc0_t = cpool.tile([P, T0, 256], f8e3)

            # PE p-state warm-up: dependency-free matmuls on a zeroed scratch
            # tile keep the tensor engine continuously busy through the first
            # input DMAs so real gate matmuls start at full clock. Results land
            # in the comm psum bank, which the first real community matmul
            # resets via its start flag.
            warm = gpool.tile([P, 512], f16, tag="warm")
            nc.gpsimd.memset(warm[:], 0.0)
            for _ in range(20):
                nc.tensor.matmul(comm[:], warm[:, 0:128], warm[:, 256:512],
                                 start=True, stop=True, skip_group_check=True)

            # ---- Phase 1: has-event nodes ----
            # Fully software-pipelined across in-order engine queues:
            #   cycle k emits: gates(k) [PE], sigma(k) [ACT], tt/npre(k) [DVE],
            #   then the LAGGED stages: tanh(k-1) [ACT] (so sigma(k) is never
            #   queued behind a tanh that waits on DVE), blend(k-1) [Pool+DVE],
            #   transposes(k-3) [PE], evac(k-3) [DVE], comm(k-4) [PE], and a
            #   few phase-0 matmuls to fill PE gaps.
            h0_state = {"next": 0, "armed": False, "dma": False}

            def emit_h0(count):
                p = h0_state["next"]
                while count > 0 and p < T0:
                    nc.tensor.matmul(comm[:], m0_t[:, p, :], inc0_t[:, p, :],
                                     start=False, stop=False)
                    p += 1; count -= 1
                h0_state["next"] = p

            # Progressive slab sizes: small first slabs start PE early; later,
            # bigger transfers hide behind compute.
            slab_edges = [0]
            for w in (512, 512, 1024, 1024):
                if slab_edges[-1] + w < H1:
                    slab_edges.append(slab_edges[-1] + w)
            while slab_edges[-1] < H1:
                slab_edges.append(min(slab_edges[-1] + DMA_SLAB, H1))

            # Stage A: gate matmuls + sigma + tt/npre for sub-slab ss.
            def stage_a(ss):
                x_s, t8_s, mT_s, inc_s, o_s, g0, s0, w = (
                    ss["x"], ss["t8"], ss["mT"], ss["inc"], ss["o"],
                    ss["g0"], ss["s0"], ss["w"])
                r_ps = rpool.tile([P, SUB], f32, tag="r")
                z_ps = zpool.tile([P, SUB], f32, tag="z")
                xn_ps = xnpool.tile([P, SUB], f32, tag="xn")
                hn_ps = hnpool.tile([P, SUB], f32, tag="hn")
                halves = []
                for h0c in range(0, w, 256):
                    cs = slice(s0 + h0c, s0 + h0c + 256)
                    rhs = [x_s[:, pl, cs].unsqueeze(1).broadcast_to([P, 2, 256])
                           for pl in range(4)]
                    rhs.append(t8_s[:, cs].unsqueeze(1).broadcast_to([KD, 2, 256]))
                    halves.append((h0c, cs, rhs))
                # r/z for BOTH halves first so sigma's input is ready early;
                # r-gate: (s,d) and (m,f) adjacent-plane pairs + t8.
                for h0c, cs, rhs in halves:
                    dest = r_ps[:, h0c:h0c + 256]
                    nc.tensor.matmul(dest, wp_t[:, 0, :, :], x_s[:, 0:2, cs],
                                     start=True, stop=False, perf_mode=DR)
                    nc.tensor.matmul(dest, wp_t[:, 1, :, :], x_s[:, 2:4, cs],
                                     start=False, stop=False, perf_mode=DR)
                    nc.tensor.matmul(dest, wp_t[0:KD, 2, :, :], rhs[4],
                                     start=False, stop=True, perf_mode=DR)
                    d_idx, pairs, planes = SEGS[0]
                    dest = z_ps[:, h0c:h0c + 256]
                    for i, (j, pl) in enumerate(zip(pairs, planes)):
                        lhs = wp_t[0:KD, j, :, :] if pl == 4 else wp_t[:, j, :, :]
                        nc.tensor.matmul(dest, lhs, rhs[pl],
                                         start=(i == 0), stop=(i == len(pairs) - 1),
                                         perf_mode=DR)
                for h0c, cs, rhs in halves:
                    for d_idx, pairs, planes in SEGS[1:]:
                        dest = (xn_ps if d_idx == 2 else hn_ps)[:, h0c:h0c + 256]
                        for i, (j, pl) in enumerate(zip(pairs, planes)):
                            lhs = wp_t[0:KD, j, :, :] if pl == 4 else wp_t[:, j, :, :]
                            nc.tensor.matmul(dest, lhs, rhs[pl],
                                             start=(i == 0), stop=(i == len(pairs) - 1),
                                             perf_mode=DR)
                rz = gpool.tile([P, 2, SUB], f16, tag="rzs")
                nc.scalar.activation(rz[:, 0, 0:w], r_ps[:, 0:w],
                                     AF.Sigmoid, scale=1.0 / WS)
                nc.scalar.activation(rz[:, 1, 0:w], z_ps[:, 0:w],
                                     AF.Sigmoid, scale=1.0 / WS)
                tt = gpool.tile([P, SUB], f16, tag="tt")
                nc.vector.tensor_tensor(tt[:, 0:w], rz[:, 0, 0:w],
                                        hn_ps[:, 0:w], A.mult)
                npre = gpool.tile([P, SUB], f16, tag="npre")
                nc.vector.tensor_tensor(npre[:, 0:w], xn_ps[:, 0:w],
                                        tt[:, 0:w], A.add)
                ss["rz"], ss["npre"] = rz, npre

            # Stage B: tanh + blend for sub-slab ss (one cycle after stage A).
            # The final sub-slab keeps its whole blend on DVE: the Pool hop
            # would add ~800ns to the drain's serial dependency chain.
            def stage_b(ss):
                w = ss["w"]
                n_t = gpool.tile([P, SUB], f16, tag="n")
                nc.scalar.activation(n_t[:, 0:w], ss["npre"][:, 0:w],
                                     AF.Tanh, scale=1.0 / WS)
                d_t = gpool.tile([P, SUB], f16, tag="d")
                nc.vector.tensor_tensor(d_t[:, 0:w],
                                        ss["mT"][:, ss["s0"]:ss["s0"] + w],
                                        n_t[:, 0:w], A.subtract)
                e_t = gpool.tile([P, SUB], f16, tag="e")
                nc.vector.tensor_tensor(e_t[:, 0:w], ss["rz"][:, 1, 0:w],
                                        d_t[:, 0:w], A.mult)
                o_eng = nc.vector if ss["final"] else nc.gpsimd
                o_eng.tensor_tensor(ss["o"][:, ss["s0"]:ss["s0"] + w],
                                    n_t[:, 0:w], e_t[:, 0:w], A.add)
                if ss["g0"] + ss["W"] >= H1:
                    # last slab: per-sub-slab stores via Pool/SWDGE (skips the
                    # HWDGE+DGE latency sitting on the drain's critical tail)
                    nc.gpsimd.dma_start(om[:, ss["g0"] + ss["s0"]:ss["g0"] + ss["s0"] + w],
                                        ss["o"][:, ss["s0"]:ss["s0"] + w])
                elif ss["last_in_slab"]:
                    nc.sync.dma_start(om[:, ss["g0"]:ss["g0"] + ss["W"]], ss["o"][:])

            # Stage C: transposes + psum evacuation (lag 3).
            def stage_c(ss):
                nt = ss["w"] // P
                tr_ps = trpool.tile([P, 4, 128], f16, tag="tr")
                nmT = gpool.tile([P, 4, 128], f16, tag="nmT")
                for k in range(nt):
                    nc.tensor.transpose(tr_ps[:, k, :],
                                        ss["o"][:, ss["s0"] + P * k:ss["s0"] + P * (k + 1)],
                                        id_t[:])
                nc.vector.tensor_copy(nmT[:, 0:nt, :], tr_ps[:, 0:nt, :])
                ss["nmT"] = nmT

            # Stage D: community matmuls (lag 4). The last node-tile carries
            # the accumulation-group stop flag (phase-0 matmuls all precede it
            # in PE program order).
            def stage_d(ss):
                for k in range(ss["w"] // P):
                    t_idx = (ss["g0"] + ss["s0"]) // P + k
                    nc.tensor.matmul(comm[:], ss["nmT"][:, k, :],
                                     ss["inc"][:, ss["s0"] // P + k, :],
                                     start=(t_idx == 0), stop=(t_idx == T1 - 1))

            subs = []
            emitted = {"b": 0, "c": 0, "d": 0}

            def pump(k):
                # Run lagged stages for cycle k of the pipeline.
                if k - 1 >= 0 and k - 1 < len(subs):
                    stage_b(subs[k - 1]); emitted["b"] = k
                if k - 3 >= 0 and k - 3 < len(subs):
                    stage_c(subs[k - 3]); emitted["c"] = k - 3
                if k - 5 >= 0 and k - 5 < len(subs):
                    stage_d(subs[k - 5]); emitted["d"] = k - 5
                    if h0_state["armed"]:
                        emit_h0(4)

            kk = 0
            pend_inc = []
            for g0, g1 in zip(slab_edges[:-1], slab_edges[1:]):
                W = g1 - g0
                x_s = xpool.tile([P, 4, W], f8, tag="x")
                nc.sync.dma_start(x_s[:], X8[:, :, g0:g0 + W])
                t8_s = xpool.tile([KD, W], f8, tag="t8")
                nc.gpsimd.dma_start(t8_s[:], t8[:, g0:g0 + W])
                mT_s = xpool.tile([P, W], f16, tag="mT")
                nc.scalar.dma_start(mT_s[:], mT[:, g0:g0 + W])
                # inc isn't consumed until the lag-4 comm stage: defer its DMA
                # one slab so early bandwidth goes to the gate streams.
                inc_s = xpool.tile([P, W // P, 256], f8e3, tag="inc")
                pend_inc.append((inc_s, g0, W))
                if len(pend_inc) > 1:
                    i_s, ig0, iW = pend_inc.pop(0)
                    nc.gpsimd.dma_start(i_s[:], inc1[:, ig0 // P:(ig0 + iW) // P, :])
                o_s = opool.tile([P, W], f16, tag="o")
                if g0 == 3072:
                    nc.scalar.dma_start(m0_t[:], m0[:])
                    nc.scalar.dma_start(inc0_t[:], inc0[:])
                    h0_state["dma"] = True
                elif g0 >= 5120:
                    h0_state["armed"] = h0_state["dma"]

                for s0 in range(0, W, SUB):
                    w = min(SUB, W - s0)
                    subs.append(dict(x=x_s, t8=t8_s, mT=mT_s, inc=inc_s, o=o_s,
                                     g0=g0, s0=s0, W=W, w=w,
                                     last_in_slab=(s0 + w >= W),
                                     final=(g0 + s0 + w >= H1)))
                    stage_a(subs[kk])
                    pump(kk)
                    kk += 1

            for i_s, ig0, iW in pend_inc:
                nc.gpsimd.dma_start(i_s[:], inc1[:, ig0 // P:(ig0 + iW) // P, :])

            if not h0_state["dma"]:  # few-slab edge case: load phase-0 now
                nc.scalar.dma_start(m0_t[:], m0[:])
                nc.scalar.dma_start(inc0_t[:], inc0[:])

            # Drain the pipeline; phase-0 remainder goes first so the final
            # community matmul (stop flag) is the true end of the psum group.
            emit_h0(T0)
            for k in range(kk, kk + 6):
                pump(k)

            cm = gpool.tile([P, 256], f32, tag="cm")
            nc.vector.tensor_copy(cm[:], comm[:])
            nc.sync.dma_start(ocm[:], cm[:])

    nc.compile()
    return nc


def _get_program(H1, H0):
    key = (H1, H0)
    if key not in _COMPILED:
        _COMPILED[key] = _build_program(H1, H0)
    return _COMPILED[key]


def _ceil_to(x, q):
    return (x + q - 1) // q * q


def kernel(src, dst, t, last_update, event_feat, src_embeds, dst_embeds,
           nodes_memory, incidence, w_time, b_time, W_ih, W_hh, b_ih, b_hh):
    src = np.asarray(src); dst = np.asarray(dst); t = np.asarray(t)
    last_update = np.asarray(last_update)
    event_feat = np.asarray(event_feat, np.float32)
    src_embeds = np.asarray(src_embeds, np.float32)
    dst_embeds = np.asarray(dst_embeds, np.float32)
    nodes_memory = np.asarray(nodes_memory, np.float32)
    incidence = np.asarray(incidence, np.float32)
    w_time = np.asarray(w_time, np.float32); b_time = np.asarray(b_time, np.float32)
    W_ih = np.asarray(W_ih, np.float32); W_hh = np.asarray(W_hh, np.float32)
    b_ih = np.asarray(b_ih, np.float32); b_hh = np.asarray(b_hh, np.float32)

    # ---- Host routing: 'last' aggregation = stable-sort scatter (index-only) ----
    src_all = np.concatenate([src, dst])
    t_all = np.concatenate([t, t])
    perm = np.argsort(t_all, kind="stable")
    win = np.zeros(N, np.int64)
    win[src_all[perm]] = perm          # ascending rank; last write = newest event
    has = np.bincount(src_all, minlength=N) > 0

    nodes1 = np.where(has)[0]
    nodes0 = np.where(~has)[0]
    n1, n0 = len(nodes1), len(nodes0)
    h1c = (n1 + NCORES - 1) // NCORES
    h0c = (n0 + NCORES - 1) // NCORES
    H1 = _ceil_to(max(h1c, 1), 256)
    H0 = _ceil_to(max(h0c, 1), 256)
    T0 = H0 // P

    # Winner-event data for has-event nodes
    w1 = win[nodes1]
    dtw = (t_all[w1] - last_update[nodes1]).astype(np.float32)
    tenc = np.cos(dtw[:, None] * w_time[None, :] + b_time[None, :])  # [n1, T]
    lt = w1 < E
    w0 = np.where(lt, w1, w1 - E)
    emb_s = np.where(lt[:, None], src_embeds[w0], dst_embeds[w0])
    emb_d = np.where(lt[:, None], dst_embeds[w0], src_embeds[w0])
    feat = event_feat[w0]

    # ---- Replicated weights: fp8 DoubleRow pairs (W8, Wlo), pre-scaled by 8 ----
    bias = (b_ih + b_hh).astype(np.float32)
    chunks = {}  # name -> [128, 384] f32 weight rows
    chunks['A'] = W_ih[0:128]
    chunks['B'] = W_ih[128:256]
    chunks['C'] = W_ih[256:384]
    Dr = np.zeros((128, 384), np.float32)
    Dr[0:T] = W_ih[384:448]
    Dr[T + 1] = bias                      # bias lane (stream row = 1.0)
    chunks['D'] = Dr
    chunks['E'] = W_hh
    # Slot layout: r (slots 0-2, no residual): [A8|B8], [E8|C8], [D8|0];
    # z (3-7), xn (8-11), hn (12): (W8, Wlo) residual pairs per chunk.
    WPa = np.zeros((P, 13, 2, 128), np.float32)

    def _hi(cname, seg):
        wseg = chunks[cname][:, 128 * seg:128 * (seg + 1)] * WS
        if seg == 1 and cname == 'D':   # z-seg D chunk carries the has-mask row
            wseg = wseg.copy()
            wseg[T] = ZBIG
        return wseg, _q8(wseg).astype(np.float32)

    WPa[:, 0, 0, :] = _hi('A', 0)[1]
    WPa[:, 0, 1, :] = _hi('B', 0)[1]
    WPa[:, 1, 0, :] = _hi('E', 0)[1]
    WPa[:, 1, 1, :] = _hi('C', 0)[1]
    WPa[:, 2, 0, :] = _hi('D', 0)[1]
    for j, (seg, cname) in enumerate([(1, c) for c in "ABCDE"] +
                                     [(2, c) for c in "ABCD"] + [(2, 'E')], start=3):
        wseg, hi = _hi(cname, seg)
        WPa[:, j, 0, :] = hi
        WPa[:, j, 1, :] = _q8(wseg - hi).astype(np.float32)
    WP_v = WPa.astype(NP_E4)

    ident = np.eye(P, dtype=np.float16)

    nc = _get_program(H1, H0)

    in_maps = []
    core_n1 = []
    for c in range(NCORES):
        i0, i1 = c * h1c, min((c + 1) * h1c, n1)
        cn1 = max(i1 - i0, 0)
        core_n1.append((i0, i1))
        sl = slice(i0, i1)
        X8c = np.zeros((P, 4, H1), NP_E4)
        X8c[:, 0, :cn1] = _q8(emb_s[sl].T)
        X8c[:, 1, :cn1] = _q8(emb_d[sl].T)
        X8c[:, 2, :cn1] = _q8(nodes_memory[nodes1[sl]].T)
        X8c[:, 3, :cn1] = _q8(feat[sl].T)
        tpl = np.zeros((T + 2, H1), np.float32)
        tpl[0:T, :cn1] = tenc[sl].T
        tpl[T, cn1:] = 1.0                # mask row: 1 on padding columns
        tpl[T + 1, :] = 1.0               # bias lane
        t8c = _q8(tpl)
        mTc = np.zeros((P, H1), np.float16)
        mTc[:, :cn1] = nodes_memory[nodes1[sl]].T.astype(np.float16)
        inc1c = np.zeros((H1 // P, P, 256), NP_E3)
        inc1c.reshape(H1, 256)[:cn1] = _q8e3(incidence[nodes1[sl]])
        inc1c = np.ascontiguousarray(inc1c.transpose(1, 0, 2))

        j0, j1 = c * h0c, min((c + 1) * h0c, n0)
        cn0 = max(j1 - j0, 0)
        sl0 = nodes0[j0:j1]
        m0c = np.zeros((H0, 128), NP_E3)
        m0c[:cn0] = _q8e3(nodes_memory[sl0])
        m0c = np.ascontiguousarray(m0c.reshape(T0, P, 128).transpose(1, 0, 2))
        inc0c = np.zeros((H0, 256), NP_E3)
        inc0c[:cn0] = _q8e3(incidence[sl0])
        inc0c = np.ascontiguousarray(inc0c.reshape(T0, P, 256).transpose(1, 0, 2))

        in_maps.append(dict(X8=X8c, t8=t8c, mT=mTc, inc1=inc1c, m0=m0c, inc0=inc0c,
                            WP=WP_v, idt=ident))

    res = run_bass_kernel_spmd(nc, in_maps, core_ids=list(range(NCORES)))

    out = np.empty((N + C, M), np.float32)
    out[:N] = nodes_memory
    comm = np.zeros((M, C), np.float64)
    for c in range(NCORES):
        i0, i1 = core_n1[c]
        if i1 > i0:
            out[nodes1[i0:i1]] = res.results[c]["om"][:, :i1 - i0].T.astype(np.float32)
        comm += res.results[c]["ocm"]
    out[N:] = comm.T.astype(np.float32)
    return out
